# revision 1
# baseline (speedup 1.0000x reference)
"""BERT-base + CRF Viterbi forward kernel for one TRN2 NeuronCore (SPMD data-parallel).

Layout strategy:
- Activations feature-major in SBUF: x_fm[kk][p=feature 128, tok] for kk in H/128 tiles,
  tok = b_local * T + t (B_loc sequences of T tokens each, B_loc*T total).
- All matmuls in plain fp32 (4 cyc/row on PE) — path correctness requires
  feats abs error <~1e-4, which rules out bf16/tf32 operands.
- LayerNorm over features via ones-matmul partition reduction + broadcast matmul.
- Attention per (batch, head): scores -> fused exp softmax -> PE transpose -> ctx.
- FFN streamed over FF/128 column chunks x token-halves, W2 accumulated in PSUM.
- CRF: ld recursion with DVE-only 3-op step (tensor_scalar add, transpose-fused
  max-reduce); psi computed afterwards in batched chunks with a bitwise-equality
  argmax; backtrace on host.
"""
from dataclasses import dataclass
from contextlib import ExitStack

import numpy as np

import concourse.bass as bass
import concourse.tile as tile
from concourse import mybir
from concourse.masks import make_identity

F32 = mybir.dt.float32
I32 = mybir.dt.int32
AX = mybir.AxisListType
OP = mybir.AluOpType
ACT = mybir.ActivationFunctionType

NEG = -10000.0
START = 21


@dataclass
class Cfg:
    B_loc: int = 4      # sequences per core
    T: int = 256        # tokens per sequence
    H: int = 768        # hidden
    L: int = 12         # layers
    NH: int = 12        # heads
    DH: int = 64        # head dim
    FF: int = 3072      # ffn dim
    NL: int = 31        # num labels
    V: int = 30522      # vocab (emb table rows)
    debug_outputs: bool = False

    @property
    def N(self):
        return self.B_loc * self.T

    @property
    def KK(self):
        return self.H // 128

    @property
    def NT(self):
        return self.T // 128

    @property
    def CC(self):
        return self.FF // 128


def _f_splits(n, c=512):
    out = []
    o = 0
    while o < n:
        s = min(c, n - o)
        out.append((o, s))
        o += s
    return out


def build(ctx: ExitStack, tc: tile.TileContext, cfg: Cfg):
    nc = tc.nc
    B, T, H, KK, NH, DH, FF, CC, NL, N = (cfg.B_loc, cfg.T, cfg.H, cfg.KK,
                                          cfg.NH, cfg.DH, cfg.FF, cfg.CC,
                                          cfg.NL, cfg.N)
    NT = cfg.NT
    NTT = N // 128
    inv_sqrt_dh = float(1.0 / np.sqrt(DH))

    # ---------------- DRAM inputs ----------------
    def din(name, shape, dt=F32):
        return nc.dram_tensor(name, shape, dt, kind="ExternalInput").ap()

    ids_d = din("ids", [N, 1], I32)
    emb_d = din("emb_table", [cfg.V, H])
    pos_d = din("pos", [T, H])
    eg_d = din("emb_g", [H])
    eb_d = din("emb_b", [H])
    Wq_d = din("Wq", [cfg.L, H, H]); bq_d = din("bq", [cfg.L, H])
    Wk_d = din("Wk", [cfg.L, H, H]); bk_d = din("bk", [cfg.L, H])
    Wv_d = din("Wv", [cfg.L, H, H]); bv_d = din("bv", [cfg.L, H])
    Wo_d = din("Wo", [cfg.L, H, H]); bo_d = din("bo", [cfg.L, H])
    g1_d = din("ln1_g", [cfg.L, H]); be1_d = din("ln1_b", [cfg.L, H])
    W1_d = din("W1", [cfg.L, H, FF]); b1_d = din("b1", [cfg.L, FF])
    W2_d = din("W2", [cfg.L, FF, H]); b2_d = din("b2", [cfg.L, H])
    g2_d = din("ln2_g", [cfg.L, H]); be2_d = din("ln2_b", [cfg.L, H])
    labW_d = din("lab_W", [H, NL])
    labb_d = din("lab_b", [NL, 1])
    trT_d = din("trT_rep", [128, 32])     # [p=(b,j), i] = tr[i, j], NEG pads
    ld0_d = din("ld0", [128, 1])
    revj_d = din("revj", [128, 32])       # 32 - j

    # ---------------- DRAM outputs ----------------
    psi_o = nc.dram_tensor("psi_out", [128, T], F32, kind="ExternalOutput").ap()
    ld_o = nc.dram_tensor("ld_out", [128, T], F32, kind="ExternalOutput").ap()
    feats_o = nc.dram_tensor("feats_out", [32, N], F32, kind="ExternalOutput").ap()
    dbg_o = None
    if cfg.debug_outputs:
        dbg_o = nc.dram_tensor("x_out", [KK * 128, N], F32,
                               kind="ExternalOutput").ap()

    # ---------------- persistent pools ----------------
    xp = ctx.enter_context(tc.tile_pool(name="xfm", bufs=2))       # x feature-major
    wp = ctx.enter_context(tc.tile_pool(name="wts", bufs=3))       # weight chunks
    bp = ctx.enter_context(tc.tile_pool(name="bias", bufs=2))      # bias columns
    sp = ctx.enter_context(tc.tile_pool(name="scratch", bufs=4))   # [128,<=1024] f32
    ap_ = ctx.enter_context(tc.tile_pool(name="attn", bufs=2))     # A / AT tiles
    qkp = ctx.enter_context(tc.tile_pool(name="qk", bufs=1))       # Q/K/V/ctx per-b
    cp = ctx.enter_context(tc.tile_pool(name="consts", bufs=1))    # persistent consts
    stp = ctx.enter_context(tc.tile_pool(name="stats", bufs=3))    # small stat rows
    hp = ctx.enter_context(tc.tile_pool(name="h1", bufs=3))        # ffn h1 chunks
    crfp = ctx.enter_context(tc.tile_pool(name="crf", bufs=2))

    # ---------------- constants ----------------
    ident = cp.tile([128, 128], F32, tag="ident")
    make_identity(nc, ident[:])
    ones_col = cp.tile([128, 1], F32, tag="ones_col")
    nc.vector.memset(ones_col[:], 1.0)
    ones_row = cp.tile([1, 128], F32, tag="ones_row")
    nc.vector.memset(ones_row[:], 1.0)
    trT = cp.tile([128, 32], F32, tag="trT")
    nc.sync.dma_start(trT[:], trT_d[:])
    revj = cp.tile([128, 32], F32, tag="revj")
    nc.sync.dma_start(revj[:], revj_d[:])
    eg_sb = cp.tile([128, KK], F32, tag="eg")
    nc.sync.dma_start(eg_sb[:], eg_d.rearrange("(kk p) -> p kk", p=128))
    eb_sb = cp.tile([128, KK], F32, tag="eb")
    nc.sync.dma_start(eb_sb[:], eb_d.rearrange("(kk p) -> p kk", p=128))
    eps_c = cp.tile([1, 1], F32, tag="eps_c")
    nc.vector.memset(eps_c[:], 1e-12)

    # ================= embedding =================
    x = [xp.tile([128, N], F32, tag=f"x{kk}", name=f"xemb{kk}") for kk in range(KK)]

    with tc.tile_pool(name="emb_ps", bufs=2, space="PSUM") as eps, \
            tc.tile_pool(name="posp", bufs=1) as posp:
        pos_sb = posp.tile([128, NT, H], F32, tag="pos")
        nc.sync.dma_start(pos_sb[:], pos_d.rearrange("(nt p) h -> p nt h", p=128))
        for r in range(NTT):
            idt = stp.tile([128, 1], I32, tag="ids")
            nc.sync.dma_start(idt[:], ids_d[r * 128:(r + 1) * 128])
            g = sp.tile([128, H], F32, tag="scr")
            nc.gpsimd.indirect_dma_start(
                out=g[:], out_offset=None, in_=emb_d[:],
                in_offset=bass.IndirectOffsetOnAxis(ap=idt[:, :1], axis=0))
            xe = sp.tile([128, H], F32, tag="scr")
            nc.vector.tensor_tensor(xe[:], g[:], pos_sb[:, r % NT], OP.add)
            for kk in range(KK):
                pt = eps.tile([128, 128], F32, tag="t")
                nc.tensor.transpose(pt[:], xe[:, kk * 128:(kk + 1) * 128],
                                    ident[:])
                nc.scalar.copy(x[kk][:, r * 128:(r + 1) * 128], pt[:])

    def layer_norm(xin, g_col, b_col, tag):
        """Feature-major LN over H: xin = list of KK tiles [128, N]."""
        with tc.tile_pool(name=f"lnps_{tag}", bufs=1, space="PSUM") as lps:
            sums = lps.tile([1, N], F32, tag="sums")
            sumsq = lps.tile([1, N], F32, tag="sumsq")
            for off, fs in _f_splits(N):
                for kk in range(KK):
                    nc.tensor.matmul(sums[:, off:off + fs], ones_col[:],
                                     xin[kk][:, off:off + fs],
                                     start=(kk == 0), stop=(kk == KK - 1))
                for kk in range(KK):
                    sq = sp.tile([128, fs], F32, tag="scr")
                    nc.scalar.square(sq[:], xin[kk][:, off:off + fs])
                    nc.tensor.matmul(sumsq[:, off:off + fs], ones_col[:],
                                     sq[:], start=(kk == 0), stop=(kk == KK - 1))
            mu = sp.tile([1, N], F32, tag="scr", name=f"mu_{tag}")
            nc.vector.tensor_scalar(mu[:], sums[:], 1.0 / H, None, OP.mult)
            var = sp.tile([1, N], F32, tag="scr", name=f"var_{tag}")
            nc.vector.tensor_tensor(var[:], mu[:], mu[:], OP.mult)
            nc.vector.tensor_scalar(sumsq[:], sumsq[:], 1.0 / H, None, OP.mult)
            nc.vector.tensor_tensor(var[:], sumsq[:], var[:], OP.subtract)
            nc.scalar.activation(var[:], var[:], ACT.Sqrt, bias=eps_c[:])
            rstd = sp.tile([1, N], F32, tag="scr", name=f"rstd_{tag}")
            nc.vector.reciprocal(rstd[:], var[:])

            mub = lps.tile([128, N], F32, tag="mub")
            rsb = lps.tile([128, N], F32, tag="rsb")
            for off, fs in _f_splits(N):
                nc.tensor.matmul(mub[:, off:off + fs], ones_row[:],
                                 mu[:, off:off + fs], start=True, stop=True)
                nc.tensor.matmul(rsb[:, off:off + fs], ones_row[:],
                                 rstd[:, off:off + fs], start=True, stop=True)
            xout = [None] * KK
            for kk in range(KK):
                t1 = sp.tile([128, N], F32, tag="scr")
                nc.vector.tensor_tensor(t1[:], xin[kk][:], mub[:], OP.subtract)
                nc.vector.tensor_tensor(t1[:], t1[:], rsb[:], OP.mult)
                o = xp.tile([128, N], F32, tag=f"x{kk}", name=f"ln_{tag}_{kk}")
                nc.vector.tensor_scalar(o[:], t1[:], g_col[:, kk:kk + 1],
                                        b_col[:, kk:kk + 1], OP.mult, OP.add)
                xout[kk] = o
        return xout

    def load_w_hh(dram, l):
        w = wp.tile([128, KK, H], F32, tag="w")
        nc.sync.dma_start(w[:], dram[l].rearrange("(kk p) c -> p kk c", p=128))
        return w

    def load_bias(dram, l, width, tag):
        b = bp.tile([128, width // 128], F32, tag=tag)
        nc.sync.dma_start(b[:], dram[l].rearrange("(kk p) -> p kk", p=128))
        return b

    x = layer_norm(x, eg_sb, eb_sb, "emb")

    # ================= encoder layers =================
    for l in range(cfg.L):
        tg = f"l{l}"
        bq = load_bias(bq_d, l, H, "bq")
        bk = load_bias(bk_d, l, H, "bk")
        bv = load_bias(bv_d, l, H, "bv")
        bo = load_bias(bo_d, l, H, "bo")
        g1 = load_bias(g1_d, l, H, "g1"); be1 = load_bias(be1_d, l, H, "be1")

        xr1 = [xp.tile([128, N], F32, tag=f"x{kk}", name=f"xr1_{tg}_{kk}")
               for kk in range(KK)]

        for b in range(B):
            ts0 = b * T
            # --- QKV projections for batch b ---
            qb = qkp.tile([128, KK, T], F32, tag="qb")
            kb = qkp.tile([128, KK, T], F32, tag="kb")
            vb = qkp.tile([128, NT, H], F32, tag="vb")
            with tc.tile_pool(name=f"qkvps_{tg}_{b}", bufs=2,
                              space="PSUM") as aps:
                wq = load_w_hh(Wq_d, l)
                wk = load_w_hh(Wk_d, l)
                for dst, w, bia in ((qb, wq, bq), (kb, wk, bk)):
                    for po in range(KK):
                        ps = aps.tile([128, T], F32, tag="proj")
                        for kk in range(KK):
                            nc.tensor.matmul(
                                ps[:], w[:, kk, po * 128:(po + 1) * 128],
                                x[kk][:, ts0:ts0 + T],
                                start=(kk == 0), stop=(kk == KK - 1))
                        nc.vector.tensor_scalar(dst[:, po], ps[:],
                                                bia[:, po:po + 1], None, OP.add)
                wv = load_w_hh(Wv_d, l)
                for pt in range(NT):
                    for off, fs in _f_splits(H):
                        ps = aps.tile([128, 512], F32, tag="vproj")
                        for kk in range(KK):
                            nc.tensor.matmul(
                                ps[:, :fs],
                                x[kk][:, ts0 + pt * 128:ts0 + (pt + 1) * 128],
                                wv[:, kk, off:off + fs],
                                start=(kk == 0), stop=(kk == KK - 1))
                        nc.scalar.copy(vb[:, pt, off:off + fs], ps[:, :fs])

            # --- attention heads ---
            ctxb = qkp.tile([128, KK, T], F32, tag="ctxb")
            with tc.tile_pool(name=f"heads_{tg}_{b}", bufs=2,
                              space="PSUM") as hps:
                for h in range(NH):
                    kk_h, p0 = divmod(h * DH, 128)
                    pa_st = [hps.tile([128, NT, 128], F32, tag=f"atps{st}",
                                      name=f"atps_{tg}_{b}_{h}_{st}")
                             for st in range(NT)]
                    for pt in range(NT):
                        sps = hps.tile([128, T], F32, tag="sps")
                        nc.tensor.matmul(
                            sps[:],
                            qb[p0:p0 + DH, kk_h, pt * 128:(pt + 1) * 128],
                            kb[p0:p0 + DH, kk_h], start=True, stop=True)
                        mx = stp.tile([128, 1], F32, tag="mx")
                        nc.vector.tensor_reduce(mx[:], sps[:], AX.X, OP.max)
                        nbias = stp.tile([128, 1], F32, tag="nbias")
                        nc.vector.tensor_scalar(nbias[:], mx[:], -inv_sqrt_dh,
                                                None, OP.mult)
                        a = ap_.tile([128, T], F32, tag="a")
                        sume = stp.tile([128, 1], F32, tag="sume")
                        nc.scalar.activation(a[:], sps[:], ACT.Exp,
                                             bias=nbias[:], scale=inv_sqrt_dh,
                                             accum_out=sume[:])
                        rec = stp.tile([128, 1], F32, tag="rec")
                        nc.vector.reciprocal(rec[:], sume[:])
                        nc.vector.tensor_scalar(a[:], a[:], rec[:], None,
                                                OP.mult)
                        for st in range(NT):
                            nc.tensor.transpose(
                                pa_st[st][:, pt],
                                a[:, st * 128:(st + 1) * 128], ident[:])
                    ats = []
                    for st in range(NT):
                        at = ap_.tile([128, NT * 128], F32, tag=f"at{st}")
                        nc.scalar.copy(at[:], pa_st[st][:].rearrange(
                            "p nt c -> p (nt c)"))
                        ats.append(at)
                    cps = hps.tile([64, T], F32, tag="cps")
                    for st in range(NT):
                        nc.tensor.matmul(cps[:],
                                         vb[:, st, h * DH:(h + 1) * DH],
                                         ats[st][:],
                                         start=(st == 0), stop=(st == NT - 1))
                    nc.vector.tensor_scalar(ctxb[p0:p0 + DH, kk_h], cps[:],
                                            bv[p0:p0 + DH, kk_h:kk_h + 1],
                                            None, OP.add)

            # --- output projection + residual for batch b ---
            with tc.tile_pool(name=f"ops_{tg}_{b}", bufs=2, space="PSUM") as ops:
                wo = load_w_hh(Wo_d, l)
                for po in range(KK):
                    ps = ops.tile([128, T], F32, tag="o")
                    for kk in range(KK):
                        nc.tensor.matmul(ps[:],
                                         wo[:, kk, po * 128:(po + 1) * 128],
                                         ctxb[:, kk],
                                         start=(kk == 0), stop=(kk == KK - 1))
                    t1 = sp.tile([128, T], F32, tag="scr")
                    nc.vector.tensor_scalar(t1[:], ps[:], bo[:, po:po + 1],
                                            None, OP.add)
                    nc.vector.tensor_tensor(xr1[po][:, ts0:ts0 + T], t1[:],
                                            x[po][:, ts0:ts0 + T], OP.add)

        y = layer_norm(xr1, g1, be1, f"{tg}a")

        # ---- FFN ----
        b1 = load_bias(b1_d, l, FF, "b1")
        b2 = load_bias(b2_d, l, H, "b2")
        g2 = load_bias(g2_d, l, H, "g2"); be2 = load_bias(be2_d, l, H, "be2")
        xr2 = [xp.tile([128, N], F32, tag=f"x{kk}", name=f"xr2_{tg}_{kk}")
               for kk in range(KK)]
        qcc = max(1, CC // 4)
        for off, fs in _f_splits(N):
            with tc.tile_pool(name=f"ffps_{tg}_{off}", bufs=1,
                              space="PSUM") as fps:
                w2ps = [fps.tile([128, fs], F32, tag=f"w2ps{po}",
                                 name=f"w2ps_{tg}_{off}_{po}")
                        for po in range(KK)]
                for c in range(CC):
                    if c % qcc == 0:
                        q0 = (c // qcc) * qcc * 128
                        w1q = wp.tile([128, KK, qcc * 128], F32, tag="w")
                        nc.sync.dma_start(
                            w1q[:], W1_d[l, :, q0:q0 + qcc * 128]
                            .rearrange("(kk p) c -> p kk c", p=128))
                        w2q = wp.tile([128, qcc, H], F32, tag="w")
                        nc.sync.dma_start(
                            w2q[:], W2_d[l, q0:q0 + qcc * 128]
                            .rearrange("(cc p) c -> p cc c", p=128))
                    h1ps = fps.tile([128, fs], F32, tag=f"h1ps{c % 2}")
                    ci = c % qcc
                    for kk in range(KK):
                        nc.tensor.matmul(h1ps[:],
                                         w1q[:, kk, ci * 128:(ci + 1) * 128],
                                         y[kk][:, off:off + fs],
                                         start=(kk == 0), stop=(kk == KK - 1))
                    h1 = hp.tile([128, fs], F32, tag="h1")
                    nc.scalar.activation(h1[:], h1ps[:], ACT.Gelu,
                                         bias=b1[:, c:c + 1])
                    for po in range(KK):
                        nc.tensor.matmul(w2ps[po][:],
                                         w2q[:, ci, po * 128:(po + 1) * 128],
                                         h1[:],
                                         start=(c == 0), stop=(c == CC - 1))
                for po in range(KK):
                    t1 = sp.tile([128, fs], F32, tag="scr")
                    nc.vector.tensor_scalar(t1[:], w2ps[po][:],
                                            b2[:, po:po + 1], None, OP.add)
                    nc.vector.tensor_tensor(xr2[po][:, off:off + fs], t1[:],
                                            y[po][:, off:off + fs], OP.add)
        x = layer_norm(xr2, g2, be2, f"{tg}b")

    # ================= label head =================
    labW = wp.tile([128, KK, NL], F32, tag="w")
    nc.sync.dma_start(labW[:], labW_d.rearrange("(kk p) c -> p kk c", p=128))
    labb = cp.tile([32, 1], F32, tag="labb")
    nc.vector.memset(labb[:], 0.0)
    nc.sync.dma_start(labb[:NL], labb_d[:])
    feats = cp.tile([32, N], F32, tag="feats")
    nc.vector.memset(feats[:], 0.0)
    with tc.tile_pool(name="labps", bufs=2, space="PSUM") as lps:
        for off, fs in _f_splits(N):
            ps = lps.tile([NL, 512], F32, tag="ps")
            for kk in range(KK):
                nc.tensor.matmul(ps[:, :fs], labW[:, kk],
                                 x[kk][:, off:off + fs],
                                 start=(kk == 0), stop=(kk == KK - 1))
            nc.vector.tensor_scalar(feats[:NL, off:off + fs], ps[:, :fs],
                                    labb[:NL], None, OP.add)
    nc.sync.dma_start(feats_o[:], feats[:])
    if dbg_o is not None:
        for kk in range(KK):
            nc.sync.dma_start(dbg_o[kk * 128:(kk + 1) * 128], x[kk][:])

    # ================= CRF =================
    Fm = cp.tile([128, T], F32, tag="Fm")
    nc.vector.memset(Fm[:], 0.0)
    for b in range(B):
        nc.sync.dma_start(Fm[b * 32:b * 32 + NL, :],
                          feats[:NL, b * T:(b + 1) * T])
    ld = cp.tile([128, T], F32, tag="ld")
    mxh = cp.tile([128, T], F32, tag="mxh")
    nc.sync.dma_start(ld[:, 0:1], ld0_d[:])
    for t in range(1, T):
        m = crfp.tile([128, 32], F32, tag="m")
        nc.vector.tensor_scalar(m[:], trT[:], ld[:, t - 1:t], None, OP.add)
        nc.vector.tensor_reduce(mxh[:, t:t + 1], m[:], AX.X, OP.max,
                                apply_transpose=True)
        nc.vector.tensor_tensor(ld[:, t:t + 1], mxh[:, t:t + 1],
                                Fm[:, t:t + 1], OP.add)
    # psi pass in chunks
    psi = cp.tile([128, T], F32, tag="psi")
    nc.vector.memset(psi[:, 0:1], 0.0)
    t0 = 1
    while t0 < T:
        tcn = min(32, T - t0)
        m = sp.tile([128, tcn, 32], F32, tag="scr")
        trT_b = trT[:].rearrange("p (o i) -> p o i", o=1).to_broadcast(
            [128, tcn, 32])
        ld_b = ld[:, t0 - 1:t0 - 1 + tcn].to_broadcast([128, tcn, 32])
        nc.vector.tensor_tensor(m[:], trT_b, ld_b, OP.add)
        mt = sp.tile([128, tcn, 32], F32, tag="scr")
        nc.vector.transpose(mt[:].rearrange("p a b -> p (a b)"),
                            m[:].rearrange("p a b -> p (a b)"))
        eq = sp.tile([128, tcn, 32], F32, tag="scr")
        mx_b = mxh[:, t0:t0 + tcn].to_broadcast([128, tcn, 32])
        nc.vector.tensor_tensor(eq[:], mt[:], mx_b, OP.is_equal)
        rev_b = revj[:].rearrange("p (o i) -> p o i", o=1).to_broadcast(
            [128, tcn, 32])
        nc.vector.tensor_tensor(eq[:], eq[:], rev_b, OP.mult)
        red = crfp.tile([128, 32], F32, tag="red")
        nc.vector.tensor_reduce(red[:, :tcn], eq[:], AX.X, OP.max)
        nc.vector.tensor_scalar(psi[:, t0:t0 + tcn], red[:, :tcn], -1.0, 32.0,
                                OP.mult, OP.add)
        t0 += tcn
    nc.sync.dma_start(psi_o[:], psi[:])
    nc.sync.dma_start(ld_o[:], ld[:])


# ======================= host-side helpers =======================

def prep_crf_consts(transitions, B=4):
    trT = np.full((32, 32), NEG, np.float32)
    tr = np.asarray(transitions, np.float32)
    nl = tr.shape[0]
    trT[:nl, :nl] = tr.T
    trT_rep = np.tile(trT, (B, 1))
    ld0 = np.full((128, 1), NEG, np.float32)
    for b in range(B):
        ld0[b * 32 + START, 0] = 0.0
    revj = np.broadcast_to((32.0 - np.arange(32)).astype(np.float32),
                           (128, 32)).copy()
    return trT_rep, ld0, revj


def host_in_map(inputs, cfg: Cfg, core):
    """Build the per-core input map from full (unsharded) reference inputs."""
    B = cfg.B_loc
    sl = slice(core * B, (core + 1) * B)
    ids = np.ascontiguousarray(
        np.asarray(inputs["input_ids"], np.int32)[sl].reshape(cfg.N, 1))
    emb = np.asarray(inputs["tok_emb"], np.float32) + \
        np.asarray(inputs["typ_emb"], np.float32)[0][None, :]
    trT_rep, ld0, revj = prep_crf_consts(np.asarray(inputs["transitions"]),
                                         cfg.B_loc)
    m = {
        "ids": ids,
        "emb_table": np.ascontiguousarray(emb),
        "pos": np.ascontiguousarray(
            np.asarray(inputs["pos_emb"], np.float32)[:cfg.T]),
        "emb_g": np.asarray(inputs["emb_ln_g"], np.float32),
        "emb_b": np.asarray(inputs["emb_ln_b"], np.float32),
        "lab_W": np.ascontiguousarray(np.asarray(inputs["lab_W"], np.float32)),
        "lab_b": np.ascontiguousarray(
            np.asarray(inputs["lab_b"], np.float32).reshape(cfg.NL, 1)),
        "trT_rep": trT_rep, "ld0": ld0, "revj": revj,
    }
    for k in ("Wq", "bq", "Wk", "bk", "Wv", "bv", "Wo", "bo", "ln1_g", "ln1_b",
              "W1", "b1", "W2", "b2", "ln2_g", "ln2_b"):
        m[k] = np.ascontiguousarray(np.asarray(inputs[k], np.float32))
    return m


def host_finish(psi_out, ld_out, cfg: Cfg):
    """psi_out/ld_out [128, T] -> (score [B], path [B, T]) for one core."""
    B, T, NL = cfg.B_loc, cfg.T, cfg.NL
    psi = psi_out.reshape(B, 32, T)   # [b, i, t]
    ldf = ld_out[:, T - 1].reshape(B, 32)[:, :NL]
    score = ldf.max(-1).astype(np.float32)
    last = ldf.argmax(-1).astype(np.int32)
    path = np.zeros((B, T), np.int32)
    path[:, T - 1] = last
    ar = np.arange(B)
    for t in range(T - 2, -1, -1):
        path[:, t] = np.rint(psi[ar, path[:, t + 1], t + 1]).astype(np.int32)
    return score, path


# ======================= SPMD runner (8 NeuronCores) =======================
from concourse import bacc as _bacc
from concourse.bass_interp import MultiCoreSim as _MultiCoreSim

N_CORES = 8
_CACHE = {}


def _get_prog():
    if "prog" not in _CACHE:
        cfg = Cfg()
        nc = _bacc.Bacc("TRN2", target_bir_lowering=False, debug=False,
                        enable_asserts=True, num_devices=N_CORES)
        with tile.TileContext(nc) as tc:
            with ExitStack() as ctx:
                build(ctx, tc, cfg)
        nc.compile()
        sim = _MultiCoreSim(nc, num_cores=N_CORES, trace=False)
        _CACHE["prog"] = (nc, cfg, sim)
    return _CACHE["prog"]


def kernel(**inputs):
    """Full (unsharded) inputs -> full (score [B], path [B, T]) outputs.

    Shards the batch over 8 NeuronCores (4 sequences each), runs the
    Bass BERT+CRF kernel on all cores, and finishes the (tiny) Viterbi
    backtrace pointer-chase on host from the device-computed psi/ld tables.
    """
    nc, cfg, sim = _get_prog()
    in_maps = [host_in_map(inputs, cfg, c) for c in range(N_CORES)]
    res = sim.run_on_hw_raw(in_maps=in_maps)
    scores, paths = [], []
    for c in range(N_CORES):
        s, p = host_finish(res.results[c]["psi_out"],
                           res.results[c]["ld_out"], cfg)
        scores.append(s)
        paths.append(p)
    return np.concatenate(scores), np.concatenate(paths)


# revision 2
# speedup vs baseline: 1.1169x; 1.1169x over previous
"""BERT-base + CRF Viterbi forward kernel for one TRN2 NeuronCore (SPMD data-parallel).

Layout strategy:
- Activations feature-major in SBUF: x_fm[kk][p=feature 128, tok] for kk in H/128 tiles,
  tok = b_local * T + t (B_loc sequences of T tokens each, B_loc*T total).
- All matmuls in plain fp32 (4 cyc/row on PE) — path correctness requires
  feats abs error <~1e-4, which rules out bf16/tf32 operands.
- LayerNorm over features via ones-matmul partition reduction + broadcast matmul.
- Attention per (batch, head): scores -> fused exp softmax -> PE transpose -> ctx.
- FFN streamed over FF/128 column chunks x token-halves, W2 accumulated in PSUM.
- CRF: ld recursion with DVE-only 3-op step (tensor_scalar add, transpose-fused
  max-reduce); psi computed afterwards in batched chunks with a bitwise-equality
  argmax; backtrace on host.
"""
from dataclasses import dataclass
from contextlib import ExitStack

import numpy as np

import concourse.bass as bass
import concourse.tile as tile
from concourse import mybir
from concourse.masks import make_identity

F32 = mybir.dt.float32
I32 = mybir.dt.int32
AX = mybir.AxisListType
OP = mybir.AluOpType
ACT = mybir.ActivationFunctionType

NEG = -10000.0
START = 21


@dataclass
class Cfg:
    B_loc: int = 4      # sequences per core
    T: int = 256        # tokens per sequence
    H: int = 768        # hidden
    L: int = 12         # layers
    NH: int = 12        # heads
    DH: int = 64        # head dim
    FF: int = 3072      # ffn dim
    NL: int = 31        # num labels
    V: int = 30522      # vocab (emb table rows)
    debug_outputs: bool = False

    @property
    def N(self):
        return self.B_loc * self.T

    @property
    def KK(self):
        return self.H // 128

    @property
    def NT(self):
        return self.T // 128

    @property
    def CC(self):
        return self.FF // 128


def _f_splits(n, c=512):
    out = []
    o = 0
    while o < n:
        s = min(c, n - o)
        out.append((o, s))
        o += s
    return out


def build(ctx: ExitStack, tc: tile.TileContext, cfg: Cfg):
    nc = tc.nc
    B, T, H, KK, NH, DH, FF, CC, NL, N = (cfg.B_loc, cfg.T, cfg.H, cfg.KK,
                                          cfg.NH, cfg.DH, cfg.FF, cfg.CC,
                                          cfg.NL, cfg.N)
    NT = cfg.NT
    NTT = N // 128
    inv_sqrt_dh = float(1.0 / np.sqrt(DH))

    # ---------------- DRAM inputs ----------------
    def din(name, shape, dt=F32):
        return nc.dram_tensor(name, shape, dt, kind="ExternalInput").ap()

    ids_d = din("ids", [N, 1], I32)
    emb_d = din("emb_table", [cfg.V, H])
    pos_d = din("pos", [T, H])
    eg_d = din("emb_g", [H])
    eb_d = din("emb_b", [H])
    Wq_d = din("Wq", [cfg.L, H, H]); bq_d = din("bq", [cfg.L, H])
    Wk_d = din("Wk", [cfg.L, H, H]); bk_d = din("bk", [cfg.L, H])
    Wv_d = din("Wv", [cfg.L, H, H]); bv_d = din("bv", [cfg.L, H])
    Wo_d = din("Wo", [cfg.L, H, H]); bo_d = din("bo", [cfg.L, H])
    g1_d = din("ln1_g", [cfg.L, H]); be1_d = din("ln1_b", [cfg.L, H])
    W1_d = din("W1", [cfg.L, H, FF]); b1_d = din("b1", [cfg.L, FF])
    W2_d = din("W2", [cfg.L, FF, H]); b2_d = din("b2", [cfg.L, H])
    g2_d = din("ln2_g", [cfg.L, H]); be2_d = din("ln2_b", [cfg.L, H])
    labW_d = din("lab_W", [H, NL])
    labb_d = din("lab_b", [NL, 1])
    trT_d = din("trT_rep", [128, 32])     # [p=(b,j), i] = tr[i, j], NEG pads
    ld0_d = din("ld0", [128, 1])
    revj_d = din("revj", [128, 32])       # 32 - j

    # ---------------- DRAM outputs ----------------
    psi_o = nc.dram_tensor("psi_out", [128, T], F32, kind="ExternalOutput").ap()
    ld_o = nc.dram_tensor("ld_out", [128, T], F32, kind="ExternalOutput").ap()
    feats_o = nc.dram_tensor("feats_out", [32, N], F32, kind="ExternalOutput").ap()
    dbg_o = None
    if cfg.debug_outputs:
        dbg_o = nc.dram_tensor("x_out", [KK * 128, N], F32,
                               kind="ExternalOutput").ap()

    # ---------------- persistent pools ----------------
    xp = ctx.enter_context(tc.tile_pool(name="xfm", bufs=2))       # x feature-major
    wp = ctx.enter_context(tc.tile_pool(name="wts", bufs=3))       # weight chunks
    bp = ctx.enter_context(tc.tile_pool(name="bias", bufs=2))      # bias columns
    sp = ctx.enter_context(tc.tile_pool(name="scratch", bufs=5))   # [128,<=1024] f32
    ap_ = ctx.enter_context(tc.tile_pool(name="attn", bufs=2))     # A / AT tiles
    qkp = ctx.enter_context(tc.tile_pool(name="qk", bufs=1))       # Q/K/V/ctx per-b
    cp = ctx.enter_context(tc.tile_pool(name="consts", bufs=1))    # persistent consts
    stp = ctx.enter_context(tc.tile_pool(name="stats", bufs=3))    # small stat rows
    hp = ctx.enter_context(tc.tile_pool(name="h1", bufs=3))        # ffn h1 chunks
    crfp = ctx.enter_context(tc.tile_pool(name="crf", bufs=2))

    # ---------------- constants ----------------
    ident = cp.tile([128, 128], F32, tag="ident")
    make_identity(nc, ident[:])
    ones_col = cp.tile([128, 1], F32, tag="ones_col")
    nc.vector.memset(ones_col[:], 1.0)
    ones_row = cp.tile([1, 128], F32, tag="ones_row")
    nc.vector.memset(ones_row[:], 1.0)
    trT = cp.tile([128, 32], F32, tag="trT")
    nc.sync.dma_start(trT[:], trT_d[:])
    revj = cp.tile([128, 32], F32, tag="revj")
    nc.sync.dma_start(revj[:], revj_d[:])
    eg_sb = cp.tile([128, KK], F32, tag="eg")
    nc.sync.dma_start(eg_sb[:], eg_d.rearrange("(kk p) -> p kk", p=128))
    eb_sb = cp.tile([128, KK], F32, tag="eb")
    nc.sync.dma_start(eb_sb[:], eb_d.rearrange("(kk p) -> p kk", p=128))
    eps_c = cp.tile([1, 1], F32, tag="eps_c")
    nc.vector.memset(eps_c[:], 1e-12)

    # ================= embedding =================
    x = [xp.tile([128, N], F32, tag=f"x{kk}", name=f"xemb{kk}") for kk in range(KK)]

    with tc.tile_pool(name="emb_ps", bufs=2, space="PSUM") as eps, \
            tc.tile_pool(name="posp", bufs=1) as posp:
        pos_sb = posp.tile([128, NT, H], F32, tag="pos")
        nc.sync.dma_start(pos_sb[:], pos_d.rearrange("(nt p) h -> p nt h", p=128))
        for r in range(NTT):
            idt = stp.tile([128, 1], I32, tag="ids")
            nc.sync.dma_start(idt[:], ids_d[r * 128:(r + 1) * 128])
            g = sp.tile([128, H], F32, tag="scr")
            nc.gpsimd.indirect_dma_start(
                out=g[:], out_offset=None, in_=emb_d[:],
                in_offset=bass.IndirectOffsetOnAxis(ap=idt[:, :1], axis=0))
            xe = sp.tile([128, H], F32, tag="scr")
            nc.vector.tensor_tensor(xe[:], g[:], pos_sb[:, r % NT], OP.add)
            for kk in range(KK):
                pt = eps.tile([128, 128], F32, tag="t")
                nc.tensor.transpose(pt[:], xe[:, kk * 128:(kk + 1) * 128],
                                    ident[:])
                nc.scalar.copy(x[kk][:, r * 128:(r + 1) * 128], pt[:])

    def layer_norm(xin, g_col, b_col, tag):
        """Feature-major LN over H: xin = list of KK tiles [128, N]."""
        with tc.tile_pool(name=f"lnps_{tag}", bufs=1, space="PSUM") as lps:
            sums = lps.tile([1, N], F32, tag="sums")
            sumsq = lps.tile([1, N], F32, tag="sumsq")
            for off, fs in _f_splits(N):
                for kk in range(KK):
                    nc.tensor.matmul(sums[:, off:off + fs], ones_col[:],
                                     xin[kk][:, off:off + fs],
                                     start=(kk == 0), stop=(kk == KK - 1))
                for kk in range(KK):
                    sq = sp.tile([128, fs], F32, tag="scr")
                    nc.scalar.square(sq[:], xin[kk][:, off:off + fs])
                    nc.tensor.matmul(sumsq[:, off:off + fs], ones_col[:],
                                     sq[:], start=(kk == 0), stop=(kk == KK - 1))
            mu = sp.tile([1, N], F32, tag="scr", name=f"mu_{tag}")
            nc.vector.tensor_scalar(mu[:], sums[:], 1.0 / H, None, OP.mult)
            var = sp.tile([1, N], F32, tag="scr", name=f"var_{tag}")
            nc.vector.tensor_tensor(var[:], mu[:], mu[:], OP.mult)
            nc.vector.tensor_scalar(sumsq[:], sumsq[:], 1.0 / H, None, OP.mult)
            nc.vector.tensor_tensor(var[:], sumsq[:], var[:], OP.subtract)
            sdt = sp.tile([1, N], F32, tag="scr", name=f"sd_{tag}")
            nc.scalar.activation(sdt[:], var[:], ACT.Sqrt, bias=eps_c[:])
            r0 = sp.tile([1, N], F32, tag="scr", name=f"r0_{tag}")
            nc.vector.reciprocal(r0[:], sdt[:])
            # Newton step: r1 = r0 * (1.5 - 0.5 * var * r0^2) — the ACT Sqrt
            # table is only ~7e-6 rel accurate, which compounds over 25 LNs.
            nc.vector.tensor_tensor(sdt[:], r0[:], r0[:], OP.mult)
            nc.vector.tensor_tensor(sdt[:], var[:], sdt[:], OP.mult)
            nc.vector.tensor_scalar(sdt[:], sdt[:], -0.5, 1.5, OP.mult, OP.add)
            rstd = sp.tile([1, N], F32, tag="scr", name=f"rstd_{tag}")
            nc.vector.tensor_tensor(rstd[:], r0[:], sdt[:], OP.mult)

            mub = lps.tile([128, N], F32, tag="mub")
            rsb = lps.tile([128, N], F32, tag="rsb")
            for off, fs in _f_splits(N):
                nc.tensor.matmul(mub[:, off:off + fs], ones_row[:],
                                 mu[:, off:off + fs], start=True, stop=True)
                nc.tensor.matmul(rsb[:, off:off + fs], ones_row[:],
                                 rstd[:, off:off + fs], start=True, stop=True)
            xout = [None] * KK
            for kk in range(KK):
                t1 = sp.tile([128, N], F32, tag="scr")
                nc.vector.tensor_tensor(t1[:], xin[kk][:], mub[:], OP.subtract)
                nc.vector.tensor_tensor(t1[:], t1[:], rsb[:], OP.mult)
                o = xp.tile([128, N], F32, tag=f"x{kk}", name=f"ln_{tag}_{kk}")
                nc.vector.tensor_scalar(o[:], t1[:], g_col[:, kk:kk + 1],
                                        b_col[:, kk:kk + 1], OP.mult, OP.add)
                xout[kk] = o
        return xout

    def load_w_hh(dram, l):
        w = wp.tile([128, KK, H], F32, tag="w")
        nc.sync.dma_start(w[:], dram[l].rearrange("(kk p) c -> p kk c", p=128))
        return w

    def load_bias(dram, l, width, tag):
        b = bp.tile([128, width // 128], F32, tag=tag)
        nc.sync.dma_start(b[:], dram[l].rearrange("(kk p) -> p kk", p=128))
        return b

    x = layer_norm(x, eg_sb, eb_sb, "emb")

    # ================= encoder layers =================
    for l in range(cfg.L):
        tg = f"l{l}"
        bq = load_bias(bq_d, l, H, "bq")
        bk = load_bias(bk_d, l, H, "bk")
        bv = load_bias(bv_d, l, H, "bv")
        bo = load_bias(bo_d, l, H, "bo")
        g1 = load_bias(g1_d, l, H, "g1"); be1 = load_bias(be1_d, l, H, "be1")

        xr1 = [xp.tile([128, N], F32, tag=f"x{kk}", name=f"xr1_{tg}_{kk}")
               for kk in range(KK)]

        for b in range(B):
            ts0 = b * T
            # --- QKV projections for batch b ---
            qb = qkp.tile([128, KK, T], F32, tag="qb")
            kb = qkp.tile([128, KK, T], F32, tag="kb")
            vb = qkp.tile([128, NT, H], F32, tag="vb")
            with tc.tile_pool(name=f"qkvps_{tg}_{b}", bufs=2,
                              space="PSUM") as aps:
                wq = load_w_hh(Wq_d, l)
                wk = load_w_hh(Wk_d, l)
                for dst, w, bia in ((qb, wq, bq), (kb, wk, bk)):
                    for po in range(KK):
                        ps = aps.tile([128, T], F32, tag="proj")
                        for kk in range(KK):
                            nc.tensor.matmul(
                                ps[:], w[:, kk, po * 128:(po + 1) * 128],
                                x[kk][:, ts0:ts0 + T],
                                start=(kk == 0), stop=(kk == KK - 1))
                        nc.vector.tensor_scalar(dst[:, po], ps[:],
                                                bia[:, po:po + 1], None, OP.add)
                wv = load_w_hh(Wv_d, l)
                for pt in range(NT):
                    for off, fs in _f_splits(H):
                        ps = aps.tile([128, 512], F32, tag="vproj")
                        for kk in range(KK):
                            nc.tensor.matmul(
                                ps[:, :fs],
                                x[kk][:, ts0 + pt * 128:ts0 + (pt + 1) * 128],
                                wv[:, kk, off:off + fs],
                                start=(kk == 0), stop=(kk == KK - 1))
                        nc.scalar.copy(vb[:, pt, off:off + fs], ps[:, :fs])

            # --- attention heads ---
            ctxb = qkp.tile([128, KK, T], F32, tag="ctxb")
            with tc.tile_pool(name=f"heads_{tg}_{b}", bufs=2,
                              space="PSUM") as hps:
                for h in range(NH):
                    kk_h, p0 = divmod(h * DH, 128)
                    pa_st = [hps.tile([128, NT, 128], F32, tag=f"atps{st}",
                                      name=f"atps_{tg}_{b}_{h}_{st}")
                             for st in range(NT)]
                    for pt in range(NT):
                        sps = hps.tile([128, T], F32, tag="sps")
                        nc.tensor.matmul(
                            sps[:],
                            qb[p0:p0 + DH, kk_h, pt * 128:(pt + 1) * 128],
                            kb[p0:p0 + DH, kk_h], start=True, stop=True)
                        mx = stp.tile([128, 1], F32, tag="mx")
                        nc.vector.tensor_reduce(mx[:], sps[:], AX.X, OP.max)
                        nbias = stp.tile([128, 1], F32, tag="nbias")
                        nc.vector.tensor_scalar(nbias[:], mx[:], -inv_sqrt_dh,
                                                None, OP.mult)
                        a = ap_.tile([128, T], F32, tag="a")
                        sume = stp.tile([128, 1], F32, tag="sume")
                        nc.scalar.activation(a[:], sps[:], ACT.Exp,
                                             bias=nbias[:], scale=inv_sqrt_dh,
                                             accum_out=sume[:])
                        rec = stp.tile([128, 1], F32, tag="rec")
                        nc.vector.reciprocal(rec[:], sume[:])
                        nc.vector.tensor_scalar(a[:], a[:], rec[:], None,
                                                OP.mult)
                        for st in range(NT):
                            nc.tensor.transpose(
                                pa_st[st][:, pt],
                                a[:, st * 128:(st + 1) * 128], ident[:])
                    ats = []
                    for st in range(NT):
                        at = ap_.tile([128, NT * 128], F32, tag=f"at{st}")
                        nc.scalar.copy(at[:], pa_st[st][:].rearrange(
                            "p nt c -> p (nt c)"))
                        ats.append(at)
                    cps = hps.tile([64, T], F32, tag="cps")
                    for st in range(NT):
                        nc.tensor.matmul(cps[:],
                                         vb[:, st, h * DH:(h + 1) * DH],
                                         ats[st][:],
                                         start=(st == 0), stop=(st == NT - 1))
                    nc.vector.tensor_scalar(ctxb[p0:p0 + DH, kk_h], cps[:],
                                            bv[p0:p0 + DH, kk_h:kk_h + 1],
                                            None, OP.add)

            # --- output projection + residual for batch b ---
            with tc.tile_pool(name=f"ops_{tg}_{b}", bufs=2, space="PSUM") as ops:
                wo = load_w_hh(Wo_d, l)
                for po in range(KK):
                    ps = ops.tile([128, T], F32, tag="o")
                    for kk in range(KK):
                        nc.tensor.matmul(ps[:],
                                         wo[:, kk, po * 128:(po + 1) * 128],
                                         ctxb[:, kk],
                                         start=(kk == 0), stop=(kk == KK - 1))
                    t1 = sp.tile([128, T], F32, tag="scr")
                    nc.vector.tensor_scalar(t1[:], ps[:], bo[:, po:po + 1],
                                            None, OP.add)
                    nc.vector.tensor_tensor(xr1[po][:, ts0:ts0 + T], t1[:],
                                            x[po][:, ts0:ts0 + T], OP.add)

        y = layer_norm(xr1, g1, be1, f"{tg}a")

        # ---- FFN ----
        b1 = load_bias(b1_d, l, FF, "b1")
        b2 = load_bias(b2_d, l, H, "b2")
        g2 = load_bias(g2_d, l, H, "g2"); be2 = load_bias(be2_d, l, H, "be2")
        xr2 = [xp.tile([128, N], F32, tag=f"x{kk}", name=f"xr2_{tg}_{kk}")
               for kk in range(KK)]
        qcc = max(1, CC // 4)
        for off, fs in _f_splits(N):
            with tc.tile_pool(name=f"ffps_{tg}_{off}", bufs=1,
                              space="PSUM") as fps:
                w2ps = [fps.tile([128, fs], F32, tag=f"w2ps{po}",
                                 name=f"w2ps_{tg}_{off}_{po}")
                        for po in range(KK)]
                for c in range(CC):
                    if c % qcc == 0:
                        q0 = (c // qcc) * qcc * 128
                        w1q = wp.tile([128, KK, qcc * 128], F32, tag="w")
                        nc.sync.dma_start(
                            w1q[:], W1_d[l, :, q0:q0 + qcc * 128]
                            .rearrange("(kk p) c -> p kk c", p=128))
                        w2q = wp.tile([128, qcc, H], F32, tag="w")
                        nc.sync.dma_start(
                            w2q[:], W2_d[l, q0:q0 + qcc * 128]
                            .rearrange("(cc p) c -> p cc c", p=128))
                    h1ps = fps.tile([128, fs], F32, tag=f"h1ps{c % 2}")
                    ci = c % qcc
                    for kk in range(KK):
                        nc.tensor.matmul(h1ps[:],
                                         w1q[:, kk, ci * 128:(ci + 1) * 128],
                                         y[kk][:, off:off + fs],
                                         start=(kk == 0), stop=(kk == KK - 1))
                    h1 = hp.tile([128, fs], F32, tag="h1")
                    nc.scalar.activation(h1[:], h1ps[:], ACT.Gelu,
                                         bias=b1[:, c:c + 1])
                    for po in range(KK):
                        nc.tensor.matmul(w2ps[po][:],
                                         w2q[:, ci, po * 128:(po + 1) * 128],
                                         h1[:],
                                         start=(c == 0), stop=(c == CC - 1))
                for po in range(KK):
                    t1 = sp.tile([128, fs], F32, tag="scr")
                    nc.vector.tensor_scalar(t1[:], w2ps[po][:],
                                            b2[:, po:po + 1], None, OP.add)
                    nc.vector.tensor_tensor(xr2[po][:, off:off + fs], t1[:],
                                            y[po][:, off:off + fs], OP.add)
        x = layer_norm(xr2, g2, be2, f"{tg}b")

    # ================= label head =================
    labW = wp.tile([128, KK, NL], F32, tag="w")
    nc.sync.dma_start(labW[:], labW_d.rearrange("(kk p) c -> p kk c", p=128))
    labb = cp.tile([32, 1], F32, tag="labb")
    nc.vector.memset(labb[:], 0.0)
    nc.sync.dma_start(labb[:NL], labb_d[:])
    feats = cp.tile([32, N], F32, tag="feats")
    nc.vector.memset(feats[:], 0.0)
    with tc.tile_pool(name="labps", bufs=2, space="PSUM") as lps:
        for off, fs in _f_splits(N):
            ps = lps.tile([NL, 512], F32, tag="ps")
            for kk in range(KK):
                nc.tensor.matmul(ps[:, :fs], labW[:, kk],
                                 x[kk][:, off:off + fs],
                                 start=(kk == 0), stop=(kk == KK - 1))
            nc.vector.tensor_scalar(feats[:NL, off:off + fs], ps[:, :fs],
                                    labb[:NL], None, OP.add)
    nc.sync.dma_start(feats_o[:], feats[:])
    if dbg_o is not None:
        for kk in range(KK):
            nc.sync.dma_start(dbg_o[kk * 128:(kk + 1) * 128], x[kk][:])

    # ================= CRF =================
    Fm = cp.tile([128, T], F32, tag="Fm")
    nc.vector.memset(Fm[:], 0.0)
    for b in range(B):
        nc.sync.dma_start(Fm[b * 32:b * 32 + NL, :],
                          feats[:NL, b * T:(b + 1) * T])
    ld = cp.tile([128, T], F32, tag="ld")
    mxh = cp.tile([128, T], F32, tag="mxh")
    nc.sync.dma_start(ld[:, 0:1], ld0_d[:])
    for t in range(1, T):
        m = crfp.tile([128, 32], F32, tag="m")
        nc.vector.tensor_scalar(m[:], trT[:], ld[:, t - 1:t], None, OP.add)
        nc.vector.tensor_reduce(mxh[:, t:t + 1], m[:], AX.X, OP.max,
                                apply_transpose=True)
        nc.vector.tensor_tensor(ld[:, t:t + 1], mxh[:, t:t + 1],
                                Fm[:, t:t + 1], OP.add)
    # psi pass in chunks
    psi = cp.tile([128, T], F32, tag="psi")
    nc.vector.memset(psi[:, 0:1], 0.0)
    t0 = 1
    while t0 < T:
        tcn = min(32, T - t0)
        m = sp.tile([128, tcn, 32], F32, tag="scr")
        trT_b = trT[:].rearrange("p (o i) -> p o i", o=1).to_broadcast(
            [128, tcn, 32])
        ld_b = ld[:, t0 - 1:t0 - 1 + tcn].to_broadcast([128, tcn, 32])
        nc.vector.tensor_tensor(m[:], trT_b, ld_b, OP.add)
        mt = sp.tile([128, tcn, 32], F32, tag="scr")
        nc.vector.transpose(mt[:].rearrange("p a b -> p (a b)"),
                            m[:].rearrange("p a b -> p (a b)"))
        eq = sp.tile([128, tcn, 32], F32, tag="scr")
        mx_b = mxh[:, t0:t0 + tcn].to_broadcast([128, tcn, 32])
        nc.vector.tensor_tensor(eq[:], mt[:], mx_b, OP.is_equal)
        rev_b = revj[:].rearrange("p (o i) -> p o i", o=1).to_broadcast(
            [128, tcn, 32])
        nc.vector.tensor_tensor(eq[:], eq[:], rev_b, OP.mult)
        red = crfp.tile([128, 32], F32, tag="red")
        nc.vector.tensor_reduce(red[:, :tcn], eq[:], AX.X, OP.max)
        nc.vector.tensor_scalar(psi[:, t0:t0 + tcn], red[:, :tcn], -1.0, 32.0,
                                OP.mult, OP.add)
        t0 += tcn
    nc.sync.dma_start(psi_o[:], psi[:])
    nc.sync.dma_start(ld_o[:], ld[:])


# ======================= host-side helpers =======================

def prep_crf_consts(transitions, B=4):
    trT = np.full((32, 32), NEG, np.float32)
    tr = np.asarray(transitions, np.float32)
    nl = tr.shape[0]
    trT[:nl, :nl] = tr.T
    trT_rep = np.tile(trT, (B, 1))
    ld0 = np.full((128, 1), NEG, np.float32)
    for b in range(B):
        ld0[b * 32 + START, 0] = 0.0
    revj = np.broadcast_to((32.0 - np.arange(32)).astype(np.float32),
                           (128, 32)).copy()
    return trT_rep, ld0, revj


def host_in_map(inputs, cfg: Cfg, core):
    """Build the per-core input map from full (unsharded) reference inputs."""
    B = cfg.B_loc
    sl = slice(core * B, (core + 1) * B)
    ids = np.ascontiguousarray(
        np.asarray(inputs["input_ids"], np.int32)[sl].reshape(cfg.N, 1))
    emb = np.asarray(inputs["tok_emb"], np.float32) + \
        np.asarray(inputs["typ_emb"], np.float32)[0][None, :]
    trT_rep, ld0, revj = prep_crf_consts(np.asarray(inputs["transitions"]),
                                         cfg.B_loc)
    m = {
        "ids": ids,
        "emb_table": np.ascontiguousarray(emb),
        "pos": np.ascontiguousarray(
            np.asarray(inputs["pos_emb"], np.float32)[:cfg.T]),
        "emb_g": np.asarray(inputs["emb_ln_g"], np.float32),
        "emb_b": np.asarray(inputs["emb_ln_b"], np.float32),
        "lab_W": np.ascontiguousarray(np.asarray(inputs["lab_W"], np.float32)),
        "lab_b": np.ascontiguousarray(
            np.asarray(inputs["lab_b"], np.float32).reshape(cfg.NL, 1)),
        "trT_rep": trT_rep, "ld0": ld0, "revj": revj,
    }
    for k in ("Wq", "bq", "Wk", "bk", "Wv", "bv", "Wo", "bo", "ln1_g", "ln1_b",
              "W1", "b1", "W2", "b2", "ln2_g", "ln2_b"):
        m[k] = np.ascontiguousarray(np.asarray(inputs[k], np.float32))
    return m


def host_finish(psi_out, ld_out, cfg: Cfg):
    """psi_out/ld_out [128, T] -> (score [B], path [B, T]) for one core."""
    B, T, NL = cfg.B_loc, cfg.T, cfg.NL
    psi = psi_out.reshape(B, 32, T)   # [b, i, t]
    ldf = ld_out[:, T - 1].reshape(B, 32)[:, :NL]
    score = ldf.max(-1).astype(np.float32)
    last = ldf.argmax(-1).astype(np.int32)
    path = np.zeros((B, T), np.int32)
    path[:, T - 1] = last
    ar = np.arange(B)
    for t in range(T - 2, -1, -1):
        path[:, t] = np.rint(psi[ar, path[:, t + 1], t + 1]).astype(np.int32)
    return score, path


# ======================= SPMD runner (8 NeuronCores) =======================
from concourse import bacc as _bacc
from concourse.bass_interp import MultiCoreSim as _MultiCoreSim

N_CORES = 8
_CACHE = {}


def _get_prog():
    if "prog" not in _CACHE:
        cfg = Cfg()
        nc = _bacc.Bacc("TRN2", target_bir_lowering=False, debug=False,
                        enable_asserts=True, num_devices=N_CORES)
        with tile.TileContext(nc) as tc:
            with ExitStack() as ctx:
                build(ctx, tc, cfg)
        nc.compile()
        sim = _MultiCoreSim(nc, num_cores=N_CORES, trace=False)
        _CACHE["prog"] = (nc, cfg, sim)
    return _CACHE["prog"]


def kernel(**inputs):
    """Full (unsharded) inputs -> full (score [B], path [B, T]) outputs.

    Shards the batch over 8 NeuronCores (4 sequences each), runs the
    Bass BERT+CRF kernel on all cores, and finishes the (tiny) Viterbi
    backtrace pointer-chase on host from the device-computed psi/ld tables.
    """
    nc, cfg, sim = _get_prog()
    in_maps = [host_in_map(inputs, cfg, c) for c in range(N_CORES)]
    res = sim.run_on_hw_raw(in_maps=in_maps)
    scores, paths = [], []
    for c in range(N_CORES):
        s, p = host_finish(res.results[c]["psi_out"],
                           res.results[c]["ld_out"], cfg)
        scores.append(s)
        paths.append(p)
    return np.concatenate(scores), np.concatenate(paths)
